# revision 6
# baseline (speedup 1.0000x reference)
"""Trainium2 Bass kernel for the Haar-mask MLP (histogram_binning).

Every Haar interval edge is a multiple of 2^-10, so the reference's masks --
and therefore the entire MLP output -- depend only on u = floor(t * 1024)
(exact in fp32).  The network collapses to a 1024x3 lookup table computed on
host from the tiny weights; the device work is: stream t, compute u, gather
LUT[u], stream out.

Gather engine: SWDGE dma_gather.  Per queue, a Q7 core-pair generates SDMA
descriptors (16 gather packets each, 128-descriptor ring) and the SDMA
engines pull 16-byte LUT rows from HBM into SBUF at ~7.6 ns/element/queue,
4 queues in parallel -- ~2x faster than the gpsimd indirect_copy ucode
(~34 Q7 cycles/element = 57 us for 16384 elements).

Raw Block structure (no TileContext): avoids the per-DMA InstIncSwdgeSem
bookkeeping (~1.3 us each on the Pool engine) that the tile framework
inserts.

Layouts per core (16384 elements, j = element ordinal):
  t_d  [128, 1024] f32: t[j] at partition 16g + j%16 (all 8 groups g),
       column j//16 -- the wrapped index layout dma_gather's tx cores read,
       replicated per 16-partition group.
  idx  int16 [128, 1024] computed on DVE in the same layout.
  dst/out [128, 128, 4] f32: element j at [j%128, j//128]; 4th word is the
       LUT row pad, dropped on host.
"""

import numpy as np

from concourse import bacc, bass, mybir
from concourse.bass_utils import run_bass_kernel_spmd
from concourse.library_config import mlp as mlp_lib

N_CORES = 8
B, T, F = 16, 8192, 3
N = B * T                    # 131072 total elements
NPC = N // N_CORES           # 16384 per neuron core
P = 128                      # SBUF partitions
S = NPC // P                 # 128 slots per partition
NBINS = 1024
ROWW = 64                    # LUT row stride: 64 f32 = 256 B (SDMA stride unit)
GE = 4                       # gathered f32 per element (16 B payload)
NQ = 4                       # SWDGE queues (ucode max)
NG = 16                      # dma_gather instructions (1024 idx each)
NCH = 2                      # DVE index-chain chunks

IMPL = "dg"
RUN_KWARGS = {}              # test harness may set {"trace": True}
LAST_RESULTS = None
_CACHE = {}


def _build_lut(W1, b1, W2, b2, W3, b3):
    """MLP output for each of the 1024 half-interval bins, fp32 math."""
    u = np.arange(NBINS)
    acc = np.zeros((NBINS, W1.shape[1]), np.float32)
    for j in range(10):
        k = u >> (10 - j)                       # floor(t * 2^j) for t in bin u
        idx = (1 << j) - 1 + k                  # level-j block offset + k
        sign = np.where((u >> (9 - j)) & 1 == 0, np.float32(1), np.float32(-1))
        acc = acc + sign[:, None] * W1[idx]
    h = np.maximum(acc + b1, np.float32(0))
    h = np.maximum(h @ W2 + b2, np.float32(0))
    return (h @ W3 + b3).astype(np.float32)     # (1024, 3)


def _dma_gather_raw(gp, out_ap, in_ap, idxs_ap, num_idxs, elem_size, elem_step,
                    queue_num):
    """gpsimd.dma_gather minus the elem_size_bytes%256 assert (non-transpose
    HBM path: only the row STRIDE must be a 256B multiple, not the payload).
    Verified on hardware with 16B payloads."""
    _in_ap = gp.lower_ap_dma(in_ap, for_custom_bir_dma=True)
    return gp.add_instruction(mybir.InstDMAGatherAnt(
        name=gp.bass.get_next_instruction_name(),
        ins=[*_in_ap, gp.lower_ap(idxs_ap),
             gp.lower_val_access(gp.to_reg(num_idxs))],
        outs=[gp.lower_ap(out_ap)],
        transpose=False, num_idxs=num_idxs, elem_size=elem_size,
        stride_bytes_256=elem_step * 4 // 256, gen_mode=0, single_packet=True,
        queue_num=queue_num, sbuf_tokens_per_rank=0, sbuf_free_dim_per_rank=0,
        sbuf_free_dim_pad_per_rank=0, sbuf_byte_offset=0))


def _build_nc_dg():
    nc = bacc.Bacc("TRN2", target_bir_lowering=False, debug=False,
                   enable_asserts=False, num_devices=N_CORES,
                   num_swdge_queues=NQ)
    f32 = mybir.dt.float32
    cols = NPC // 16                             # 1024 idx columns
    t_d = nc.dram_tensor("t", [P, cols], f32, kind="ExternalInput")
    lut_d = nc.dram_tensor("lut", [NBINS, ROWW], f32, kind="ExternalInput")
    out_d = nc.dram_tensor("out", [P, S, GE], f32, kind="ExternalOutput")

    ipg = NPC // NG                              # indices per gather inst
    cpg = cols // NG                             # idx columns per gather
    spg = S // NG                                # dst slots per gather
    ccc = cols // NCH                            # chain columns per chunk
    gpr = NG // NQ                               # gathers per queue (rounds)

    with (
        nc.Block() as block,
        nc.sbuf_tensor("t_sb", [P, cols], f32) as t_sb,
        nc.sbuf_tensor("uf", [P, cols], f32) as uf,
        nc.sbuf_tensor("ii", [P, cols], mybir.dt.int32) as ii,
        nc.sbuf_tensor("fb", [P, cols], f32) as fb,
        nc.sbuf_tensor("adj", [P, cols], f32) as adj,
        nc.sbuf_tensor("idx", [P, cols], mybir.dt.int16) as idx,
        nc.sbuf_tensor("dst", [P, S, GE], f32) as dst,
        nc.semaphore("io") as io,
        nc.semaphore("vs") as vs,
        nc.semaphore("q0") as q0,
        nc.semaphore("q1") as q1,
        nc.semaphore("q2") as q2,
        nc.semaphore("q3") as q3,
        nc.semaphore("ou") as ou,
    ):
        qsems = [q0, q1, q2, q3]

        @block.sync
        def _(s):
            s.dma_start(t_sb[0:32, :], t_d[0:32, :]).then_inc(io, 16)
            s.dma_start(t_sb[64:96, :], t_d[64:96, :]).then_inc(io, 16)

        @block.scalar
        def _(s):
            s.dma_start(t_sb[32:64, :], t_d[32:64, :]).then_inc(io, 16)
            s.dma_start(t_sb[96:128, :], t_d[96:128, :]).then_inc(io, 16)

        @block.vector
        def _(v):
            v.wait_ge(io, 64)
            for c in range(NCH):
                sl = slice(c * ccc, (c + 1) * ccc)
                # exact floor(t*1024): round-to-int (any rounding mode), then
                # subtract 1 wherever the rounded value exceeds the true value
                v.tensor_scalar(uf[:, sl], t_sb[:, sl], 1024.0, None,
                                mybir.AluOpType.mult)
                v.tensor_copy(ii[:, sl], uf[:, sl])
                v.tensor_copy(fb[:, sl], ii[:, sl])
                v.tensor_tensor(adj[:, sl], fb[:, sl], uf[:, sl],
                                mybir.AluOpType.is_gt)
                v.tensor_sub(fb[:, sl], fb[:, sl], adj[:, sl])
                v.tensor_scalar(idx[:, sl], fb[:, sl], 1023.0, None,
                                mybir.AluOpType.min).then_inc(vs, 1)

        @block.gpsimd
        def _(gp):
            gp.load_library(mlp_lib)
            for k in range(NG):
                need = (k * cpg + cpg + ccc - 1) // ccc   # chain chunks needed
                gp.wait_ge(vs, min(need, NCH))
                _dma_gather_raw(
                    gp, dst[:, k * spg:(k + 1) * spg, :], lut_d[:, 0:GE],
                    idx[:, k * cpg:(k + 1) * cpg], ipg, GE, ROWW,
                    k % NQ).then_inc(qsems[k % NQ], 16)

        @block.sync
        def _(s):
            for r in range(gpr):
                for q in range(NQ):
                    s.wait_ge(qsems[q], 16 * (r + 1))
                sl = slice(r * spg * NQ, (r + 1) * spg * NQ)
                s.dma_start(out_d.ap()[:, sl, :], dst[:, sl, :]).then_inc(ou, 16)
            s.wait_ge(ou, 16 * gpr)
    nc.compile()
    return nc


def _host_inputs(t, lut):
    tf = np.ascontiguousarray(np.asarray(t, np.float32)).reshape(-1)
    # wrapped layout: element j -> partition j%16, column j//16, x8 groups
    tw = tf.reshape(N_CORES, NPC // 16, 16).transpose(0, 2, 1)  # [m, 16, cols]
    tperm = np.tile(tw, (1, 8, 1))                              # [m, 128, cols]
    lutp = np.zeros((NBINS, ROWW), np.float32)
    lutp[:, :F] = lut
    return tperm, lutp


def kernel(t, W1, b1, W2, b2, W3, b3):
    global LAST_RESULTS
    key = ("nc", IMPL)
    if key not in _CACHE:
        _CACHE[key] = _build_nc_dg()
    nc = _CACHE[key]

    lut = _build_lut(np.asarray(W1, np.float32), np.asarray(b1, np.float32),
                     np.asarray(W2, np.float32), np.asarray(b2, np.float32),
                     np.asarray(W3, np.float32), np.asarray(b3, np.float32))
    tperm, lutp = _host_inputs(t, lut)
    in_maps = [{"t": np.ascontiguousarray(tperm[m]), "lut": lutp}
               for m in range(N_CORES)]

    res = run_bass_kernel_spmd(nc, in_maps, list(range(N_CORES)), **RUN_KWARGS)
    LAST_RESULTS = res
    # out[p, s] = element s*128 + p; 4th gathered word is pad
    outs = [res.results[m]["out"][:, :, :F].transpose(1, 0, 2).reshape(NPC, F)
            for m in range(N_CORES)]
    return np.concatenate(outs, axis=0).reshape(B, T, F).astype(np.float32)


# revision 24
# speedup vs baseline: 1.0977x; 1.0977x over previous
"""Trainium2 Bass kernel for the Haar-mask MLP (histogram_binning).

Every Haar interval edge is a multiple of 2^-10, so the reference's masks --
and therefore the entire MLP output -- depend only on u = floor(t * 1024)
(exact in fp32).  The network collapses to a 1024x3 lookup table computed on
host from the tiny weights; the device work is: stream t, compute u, gather
LUT[u], stream out.

Gather engine: SWDGE dma_gather.  Q7 core-pairs (one per queue, 4 queues)
generate SDMA descriptors (16 gather packets each, 128-descriptor ring) and
the SDMA engines pull 16-byte LUT rows from 256B-strided HBM rows.  Measured
~2.6 ns/element aggregate -- 2.2x the old gpsimd indirect_copy ucode path
(~3.5 ns/elem) once instruction gen, ring stalls and tail are accounted.
512-idx instructions keep the per-queue rings shallow (33 descriptors) so
ring-full stalls inside the ucode stay short and the final drain tail is
~5 us instead of ~10.

Raw Block structure (no TileContext): avoids the per-DMA InstIncSwdgeSem
bookkeeping (~1.3 us each) the tile framework inserts.

Layouts per core (16384 elements, j = element ordinal):
  t_d  [128, 1024] f32: t[j] at partition 16g + j%16 (all 8 groups g),
       column j//16 -- the wrapped index layout whose per-group replicas the
       dma_gather tx cores read.  DMA'd in 4 column-quarters so the index
       chain starts after the first quarter lands.
  idx  uint16 [128, 1024] computed on DVE in 160-column chunks (the final
       chunk is 224 columns: narrow trailing uint16 chunks mis-write).
  dst/out [128, 128, 4] f32: element j at [j%128, j//128]; 4th word pad.
"""

import numpy as np
from contextlib import ExitStack

from concourse import bacc, mybir
from concourse.bass_utils import run_bass_kernel_spmd
from concourse.library_config import mlp as mlp_lib

N_CORES = 8
B, T, F = 16, 8192, 3
N = B * T                    # 131072 total elements
NPC = N // N_CORES           # 16384 per neuron core
P = 128
S = NPC // P                 # 128 slots per partition
NBINS = 1024
ROWW = 64                    # LUT row stride: 64 f32 = 256 B (SDMA stride unit)
GE = 4                       # gathered f32 per element (16 B payload)
NQ = 4                       # SWDGE queues (ucode max)
NG = 32                      # dma_gather instructions (512 idx each)
COLS = NPC // 16             # 1024 idx columns
CHB = [(0, 160), (160, 320), (320, 480), (480, 640), (640, 800), (800, COLS)]

IMPL = "dg"
RUN_KWARGS = {}
LAST_RESULTS = None
_CACHE = {}


def _build_lut(W1, b1, W2, b2, W3, b3):
    """MLP output for each of the 1024 half-interval bins, fp32 math."""
    u = np.arange(NBINS)
    acc = np.zeros((NBINS, W1.shape[1]), np.float32)
    for j in range(10):
        k = u >> (10 - j)                       # floor(t * 2^j) for t in bin u
        idx = (1 << j) - 1 + k                  # level-j block offset + k
        sign = np.where((u >> (9 - j)) & 1 == 0, np.float32(1), np.float32(-1))
        acc = acc + sign[:, None] * W1[idx]
    h = np.maximum(acc + b1, np.float32(0))
    h = np.maximum(h @ W2 + b2, np.float32(0))
    return (h @ W3 + b3).astype(np.float32)     # (1024, 3)


def _dma_gather_raw(gp, out_ap, in_ap, idxs_ap, num_idxs, elem_size, elem_step,
                    queue_num):
    """gpsimd.dma_gather minus the elem_size_bytes%256 assert (non-transpose
    HBM path: only the row STRIDE must be a 256B multiple, not the payload).
    Verified on hardware with 16B payloads."""
    _in_ap = gp.lower_ap_dma(in_ap, for_custom_bir_dma=True)
    return gp.add_instruction(mybir.InstDMAGatherAnt(
        name=gp.bass.get_next_instruction_name(),
        ins=[*_in_ap, gp.lower_ap(idxs_ap),
             gp.lower_val_access(gp.to_reg(num_idxs))],
        outs=[gp.lower_ap(out_ap)],
        transpose=False, num_idxs=num_idxs, elem_size=elem_size,
        stride_bytes_256=elem_step * 4 // 256, gen_mode=0, single_packet=True,
        queue_num=queue_num, sbuf_tokens_per_rank=0, sbuf_free_dim_per_rank=0,
        sbuf_free_dim_pad_per_rank=0, sbuf_byte_offset=0))


def _build_nc():
    nc = bacc.Bacc("TRN2", target_bir_lowering=False, debug=False,
                   enable_asserts=False, num_devices=N_CORES,
                   num_swdge_queues=NQ)
    f32 = mybir.dt.float32
    t_d = nc.dram_tensor("t", [P, COLS], f32, kind="ExternalInput")
    lut_d = nc.dram_tensor("lut", [NBINS, ROWW], f32, kind="ExternalInput")
    out_d = nc.dram_tensor("out", [P, S, GE], f32, kind="ExternalOutput")

    ipg = NPC // NG                              # indices per gather: 512
    cpg = COLS // NG                             # idx columns per gather: 16
    spg = S // NG                                # dst slots per gather: 4
    gpr = NG // NQ                               # rounds: 8
    tq = COLS // 4                               # t DMA column quarter

    with nc.Block() as block, ExitStack() as ctx:
        sb = lambda name, shape, dt: ctx.enter_context(
            nc.sbuf_tensor(name, shape, dt))
        sem = lambda name: ctx.enter_context(nc.semaphore(name))
        t_sb = sb("t_sb", [P, COLS], f32)
        uf = sb("uf", [P, COLS], f32)
        ii = sb("ii", [P, COLS], mybir.dt.int32)
        fb = sb("fb", [P, COLS], f32)
        adj = sb("adj", [P, COLS], f32)
        idx = sb("idx", [P, COLS], mybir.dt.uint16)
        dst = sb("dst", [P, S, GE], f32)
        io, vs, ou = sem("io"), sem("vs"), sem("ou")
        qsems = [sem(f"q{q}") for q in range(NQ)]

        @block.sync
        def _(s):
            s.dma_start(t_sb[:, 0 * tq:1 * tq], t_d[:, 0 * tq:1 * tq]
                        ).then_inc(io, 16)
            s.dma_start(t_sb[:, 2 * tq:3 * tq], t_d[:, 2 * tq:3 * tq]
                        ).then_inc(io, 16)

        @block.scalar
        def _(s):
            s.dma_start(t_sb[:, 1 * tq:2 * tq], t_d[:, 1 * tq:2 * tq]
                        ).then_inc(io, 16)
            s.dma_start(t_sb[:, 3 * tq:4 * tq], t_d[:, 3 * tq:4 * tq]
                        ).then_inc(io, 16)

        @block.vector
        def _(v):
            done = 0
            for c0, c1 in CHB:
                # wait for the t column-quarters covering [c0, c1).
                # quarters land in order 1,2 (sync/scalar first DMAs), 3,4;
                # io counts 16 per quarter but quarter completion order is
                # q0(sync), q1(scalar) concurrently then q2, q3 -- wait for
                # all quarters overlapping this chunk conservatively.
                nq_need = (c1 + tq - 1) // tq
                if nq_need > done:
                    v.wait_ge(io, 16 * ((4 if nq_need > 2 else 2)
                                        if nq_need > 1 else 2))
                    done = nq_need
                sl = slice(c0, c1)
                # exact floor(t*1024): round-to-int (any rounding mode), then
                # subtract 1 wherever the rounded value exceeds the true value
                v.tensor_scalar(uf[:, sl], t_sb[:, sl], 1024.0, None,
                                mybir.AluOpType.mult)
                v.tensor_copy(ii[:, sl], uf[:, sl])
                v.tensor_copy(fb[:, sl], ii[:, sl])
                v.tensor_tensor(adj[:, sl], fb[:, sl], uf[:, sl],
                                mybir.AluOpType.is_gt)
                v.tensor_sub(fb[:, sl], fb[:, sl], adj[:, sl])
                v.tensor_scalar(idx[:, sl], fb[:, sl], 1023.0, 0.0,
                                mybir.AluOpType.min,
                                mybir.AluOpType.max).then_inc(vs, 1)

        @block.gpsimd
        def _(gp):
            gp.load_library(mlp_lib)
            for k in range(NG):
                # chain chunk covering idx columns [k*cpg, (k+1)*cpg)
                end = (k + 1) * cpg
                need = next(i + 1 for i, (c0, c1) in enumerate(CHB)
                            if c1 >= end)
                gp.wait_ge(vs, need)
                _dma_gather_raw(
                    gp, dst[:, k * spg:(k + 1) * spg, :], lut_d[:, 0:GE],
                    idx[:, k * cpg:(k + 1) * cpg], ipg, GE, ROWW,
                    k % NQ).then_inc(qsems[k % NQ], 16)

        @block.sync
        def _(s):
            for r in range(gpr):
                for q in range(NQ):
                    s.wait_ge(qsems[q], 16 * (r + 1))
                sl = slice(r * spg * NQ, (r + 1) * spg * NQ)
                s.dma_start(out_d.ap()[:, sl, :], dst[:, sl, :]).then_inc(ou, 16)
            s.wait_ge(ou, 16 * gpr)
    nc.compile()
    return nc


def _host_inputs(t, lut):
    tf = np.ascontiguousarray(np.asarray(t, np.float32)).reshape(-1)
    # wrapped layout: element j -> partition j%16, column j//16, x8 groups
    tw = tf.reshape(N_CORES, COLS, 16).transpose(0, 2, 1)       # [m, 16, cols]
    tperm = np.tile(tw, (1, 8, 1))                              # [m, 128, cols]
    lutp = np.zeros((NBINS, ROWW), np.float32)
    lutp[:, :F] = lut
    return tperm, lutp


def kernel(t, W1, b1, W2, b2, W3, b3):
    global LAST_RESULTS
    key = ("nc", IMPL)
    if key not in _CACHE:
        _CACHE[key] = _build_nc()
    nc = _CACHE[key]

    lut = _build_lut(np.asarray(W1, np.float32), np.asarray(b1, np.float32),
                     np.asarray(W2, np.float32), np.asarray(b2, np.float32),
                     np.asarray(W3, np.float32), np.asarray(b3, np.float32))
    tperm, lutp = _host_inputs(t, lut)
    in_maps = [{"t": np.ascontiguousarray(tperm[m]), "lut": lutp}
               for m in range(N_CORES)]

    res = run_bass_kernel_spmd(nc, in_maps, list(range(N_CORES)), **RUN_KWARGS)
    LAST_RESULTS = res
    # out[p, s] = element s*128 + p; 4th gathered word is pad
    outs = [res.results[m]["out"][:, :, :F].transpose(1, 0, 2).reshape(NPC, F)
            for m in range(N_CORES)]
    return np.concatenate(outs, axis=0).reshape(B, T, F).astype(np.float32)


# revision 25
# speedup vs baseline: 1.1849x; 1.0795x over previous
"""Trainium2 Bass kernel for the Haar-mask MLP (histogram_binning).

Every Haar interval edge is a multiple of 2^-10, so the reference's masks --
and therefore the entire MLP output -- depend only on u = floor(t * 1024)
(exact in fp32).  The network collapses to a 1024x3 lookup table computed on
host from the tiny weights; the device work is: stream t, compute u, gather
LUT[u], stream out.

Gather engine: SWDGE dma_gather.  Q7 core-pairs (one per queue, 4 queues)
generate SDMA descriptors (16 gather packets each, 128-descriptor ring) and
the SDMA engines pull 16-byte LUT rows from 256B-strided HBM rows.  Measured
~2.6 ns/element aggregate -- 2.2x the old gpsimd indirect_copy ucode path
(~3.5 ns/elem) once instruction gen, ring stalls and tail are accounted.
512-idx instructions keep the per-queue rings shallow (33 descriptors) so
ring-full stalls inside the ucode stay short and the final drain tail is
~5 us instead of ~10.

Raw Block structure (no TileContext): avoids the per-DMA InstIncSwdgeSem
bookkeeping (~1.3 us each) the tile framework inserts.

Layouts per core (16384 elements, j = element ordinal):
  t_d  [128, 1024] f32: t[j] at partition 16g + j%16 (all 8 groups g),
       column j//16 -- the wrapped index layout whose per-group replicas the
       dma_gather tx cores read.  DMA'd in 4 column-quarters so the index
       chain starts after the first quarter lands.
  idx  uint16 [128, 1024] computed on DVE in 160-column chunks (the final
       chunk is 224 columns: narrow trailing uint16 chunks mis-write).
  dst/out [128, 128, 4] f32: element j at [j%128, j//128]; 4th word pad.
"""

import numpy as np
from contextlib import ExitStack

from concourse import bacc, mybir
from concourse.bass_utils import run_bass_kernel_spmd
from concourse.library_config import mlp as mlp_lib

N_CORES = 8
B, T, F = 16, 8192, 3
N = B * T                    # 131072 total elements
NPC = N // N_CORES           # 16384 per neuron core
P = 128
S = NPC // P                 # 128 slots per partition
NBINS = 1024
ROWW = 64                    # LUT row stride: 64 f32 = 256 B (SDMA stride unit)
GE = 4                       # gathered f32 per element (16 B payload)
NQ = 4                       # SWDGE queues (ucode max)
NG = 16                      # dma_gather instructions (1024 idx each)
COLS = NPC // 16             # 1024 idx columns
CHB = [(0, 160), (160, 320), (320, 480), (480, 640), (640, 800), (800, COLS)]

IMPL = "dg"
RUN_KWARGS = {}
LAST_RESULTS = None
_CACHE = {}


def _build_lut(W1, b1, W2, b2, W3, b3):
    """MLP output for each of the 1024 half-interval bins, fp32 math."""
    u = np.arange(NBINS)
    acc = np.zeros((NBINS, W1.shape[1]), np.float32)
    for j in range(10):
        k = u >> (10 - j)                       # floor(t * 2^j) for t in bin u
        idx = (1 << j) - 1 + k                  # level-j block offset + k
        sign = np.where((u >> (9 - j)) & 1 == 0, np.float32(1), np.float32(-1))
        acc = acc + sign[:, None] * W1[idx]
    h = np.maximum(acc + b1, np.float32(0))
    h = np.maximum(h @ W2 + b2, np.float32(0))
    return (h @ W3 + b3).astype(np.float32)     # (1024, 3)


def _dma_gather_raw(gp, out_ap, in_ap, idxs_ap, num_idxs, elem_size, elem_step,
                    queue_num):
    """gpsimd.dma_gather minus the elem_size_bytes%256 assert (non-transpose
    HBM path: only the row STRIDE must be a 256B multiple, not the payload).
    Verified on hardware with 16B payloads."""
    _in_ap = gp.lower_ap_dma(in_ap, for_custom_bir_dma=True)
    return gp.add_instruction(mybir.InstDMAGatherAnt(
        name=gp.bass.get_next_instruction_name(),
        ins=[*_in_ap, gp.lower_ap(idxs_ap),
             gp.lower_val_access(gp.to_reg(num_idxs))],
        outs=[gp.lower_ap(out_ap)],
        transpose=False, num_idxs=num_idxs, elem_size=elem_size,
        stride_bytes_256=elem_step * 4 // 256, gen_mode=0, single_packet=True,
        queue_num=queue_num, sbuf_tokens_per_rank=0, sbuf_free_dim_per_rank=0,
        sbuf_free_dim_pad_per_rank=0, sbuf_byte_offset=0))


def _build_nc():
    nc = bacc.Bacc("TRN2", target_bir_lowering=False, debug=False,
                   enable_asserts=False, num_devices=N_CORES,
                   num_swdge_queues=NQ)
    f32 = mybir.dt.float32
    t_d = nc.dram_tensor("t", [P, COLS], f32, kind="ExternalInput")
    lut_d = nc.dram_tensor("lut", [NBINS, ROWW], f32, kind="ExternalInput")
    out_d = nc.dram_tensor("out", [P, S, GE], f32, kind="ExternalOutput")

    ipg = NPC // NG                              # indices per gather: 512
    cpg = COLS // NG                             # idx columns per gather: 16
    spg = S // NG                                # dst slots per gather: 4
    gpr = NG // NQ                               # rounds: 8
    tq = COLS // 4                               # t DMA column quarter

    with nc.Block() as block, ExitStack() as ctx:
        sb = lambda name, shape, dt: ctx.enter_context(
            nc.sbuf_tensor(name, shape, dt))
        sem = lambda name: ctx.enter_context(nc.semaphore(name))
        t_sb = sb("t_sb", [P, COLS], f32)
        uf = sb("uf", [P, COLS], f32)
        ii = sb("ii", [P, COLS], mybir.dt.int32)
        fb = sb("fb", [P, COLS], f32)
        adj = sb("adj", [P, COLS], f32)
        idx = sb("idx", [P, COLS], mybir.dt.uint16)
        dst = sb("dst", [P, S, GE], f32)
        io, vs, ou = sem("io"), sem("vs"), sem("ou")
        qsems = [sem(f"q{q}") for q in range(NQ)]

        @block.sync
        def _(s):
            s.dma_start(t_sb[:, 0 * tq:1 * tq], t_d[:, 0 * tq:1 * tq]
                        ).then_inc(io, 16)
            s.dma_start(t_sb[:, 2 * tq:3 * tq], t_d[:, 2 * tq:3 * tq]
                        ).then_inc(io, 16)

        @block.scalar
        def _(s):
            s.dma_start(t_sb[:, 1 * tq:2 * tq], t_d[:, 1 * tq:2 * tq]
                        ).then_inc(io, 16)
            s.dma_start(t_sb[:, 3 * tq:4 * tq], t_d[:, 3 * tq:4 * tq]
                        ).then_inc(io, 16)

        @block.vector
        def _(v):
            done = 0
            for c0, c1 in CHB:
                # wait for the t column-quarters covering [c0, c1).
                # quarters land in order 1,2 (sync/scalar first DMAs), 3,4;
                # io counts 16 per quarter but quarter completion order is
                # q0(sync), q1(scalar) concurrently then q2, q3 -- wait for
                # all quarters overlapping this chunk conservatively.
                nq_need = (c1 + tq - 1) // tq
                if nq_need > done:
                    v.wait_ge(io, 16 * ((4 if nq_need > 2 else 2)
                                        if nq_need > 1 else 2))
                    done = nq_need
                sl = slice(c0, c1)
                # exact floor(t*1024): round-to-int (any rounding mode), then
                # subtract 1 wherever the rounded value exceeds the true value
                v.tensor_scalar(uf[:, sl], t_sb[:, sl], 1024.0, None,
                                mybir.AluOpType.mult)
                v.tensor_copy(ii[:, sl], uf[:, sl])
                v.tensor_copy(fb[:, sl], ii[:, sl])
                v.tensor_tensor(adj[:, sl], fb[:, sl], uf[:, sl],
                                mybir.AluOpType.is_gt)
                v.tensor_sub(fb[:, sl], fb[:, sl], adj[:, sl])
                v.tensor_scalar(idx[:, sl], fb[:, sl], 1023.0, 0.0,
                                mybir.AluOpType.min,
                                mybir.AluOpType.max).then_inc(vs, 1)

        @block.gpsimd
        def _(gp):
            gp.load_library(mlp_lib)
            for k in range(NG):
                # chain chunk covering idx columns [k*cpg, (k+1)*cpg)
                end = (k + 1) * cpg
                need = next(i + 1 for i, (c0, c1) in enumerate(CHB)
                            if c1 >= end)
                gp.wait_ge(vs, need)
                _dma_gather_raw(
                    gp, dst[:, k * spg:(k + 1) * spg, :], lut_d[:, 0:GE],
                    idx[:, k * cpg:(k + 1) * cpg], ipg, GE, ROWW,
                    k % NQ).then_inc(qsems[k % NQ], 16)

        @block.sync
        def _(s):
            for r in range(gpr):
                for q in range(NQ):
                    s.wait_ge(qsems[q], 16 * (r + 1))
                sl = slice(r * spg * NQ, (r + 1) * spg * NQ)
                s.dma_start(out_d.ap()[:, sl, :], dst[:, sl, :]).then_inc(ou, 16)
            s.wait_ge(ou, 16 * gpr)
    nc.compile()
    return nc


def _host_inputs(t, lut):
    tf = np.ascontiguousarray(np.asarray(t, np.float32)).reshape(-1)
    # wrapped layout: element j -> partition j%16, column j//16, x8 groups
    tw = tf.reshape(N_CORES, COLS, 16).transpose(0, 2, 1)       # [m, 16, cols]
    tperm = np.tile(tw, (1, 8, 1))                              # [m, 128, cols]
    lutp = np.zeros((NBINS, ROWW), np.float32)
    lutp[:, :F] = lut
    return tperm, lutp


def kernel(t, W1, b1, W2, b2, W3, b3):
    global LAST_RESULTS
    key = ("nc", IMPL)
    if key not in _CACHE:
        _CACHE[key] = _build_nc()
    nc = _CACHE[key]

    lut = _build_lut(np.asarray(W1, np.float32), np.asarray(b1, np.float32),
                     np.asarray(W2, np.float32), np.asarray(b2, np.float32),
                     np.asarray(W3, np.float32), np.asarray(b3, np.float32))
    tperm, lutp = _host_inputs(t, lut)
    in_maps = [{"t": np.ascontiguousarray(tperm[m]), "lut": lutp}
               for m in range(N_CORES)]

    res = run_bass_kernel_spmd(nc, in_maps, list(range(N_CORES)), **RUN_KWARGS)
    LAST_RESULTS = res
    # out[p, s] = element s*128 + p; 4th gathered word is pad
    outs = [res.results[m]["out"][:, :, :F].transpose(1, 0, 2).reshape(NPC, F)
            for m in range(N_CORES)]
    return np.concatenate(outs, axis=0).reshape(B, T, F).astype(np.float32)
